# revision 18
# baseline (speedup 1.0000x reference)
"""Trainium2 kernel for nn_MemoryMolecular retrieval_knn.

reference:
    logits = x @ feature_queue.T          # [2048, 65536] fp32
    pos = rep_queue[argmax(logits, -1)]; neg = rep_queue[argmin(logits, -1)]

Strategy: shard K across the 8 NeuronCores (8192 columns each).  The host
quantizes x / feature_queue to fp8e4m3; each core computes its logit shard
with DoubleRow fp8 matmuls.  PSUM chunks [128, QW] are drained to SBUF as fp8
by the Scalar and Vector engines (split at chunk granularity: ScalarE drains
(172+FD)/1.2GHz, DVE (120+FD)/0.96GHz per chunk, balanced at ~0.54), then
shipped to DRAM per q-group on alternating DMA queues.  The host dequantizes
and exactly rescores (fp64) every candidate within a margin that bounds the
total quantization error, recovering the exact fp32 argmax/argmin before
gathering rep_queue rows.

Per-core steady state (measured by repetition slope at R=1025, min-of-25):
the 512 DoubleRow matmuls stream in ~114 us (vs 109 us theoretical at
157 TF/s fp8), drains (~70 us across both engines) and the 16.7 MB logits
DMA (~47 us) hide underneath; full body ~135 us/iteration.  Repeat builds
unroll the body 8x inside the For_i hardware loop because each For_i
iteration carries an all-engine barrier + semaphore-reset block (~10-20 us
exposed otherwise).
"""
import numpy as np
import concourse.bacc as bacc
import concourse.mybir as mybir
import concourse.tile as tile
from concourse.bass_utils import run_bass_kernel_spmd

B, K, F = 2048, 65536, 512
NCORES = 8
KS = K // NCORES          # 8192 columns per core
NF = F // 128             # 4 contraction blocks of 128
NT = B // 128             # 16 row tiles
QW = 1024                 # columns per q-group / psum chunk width
NQ = KS // QW             # 8
CW = 512                  # per-matmul moving width (1 psum bank)

E4 = mybir.dt.float8e4
MARGIN = 24.0             # host rescore margin, covers fp8 in+out quantization

_nc_cache = None


def _chunk_engines(n, act_frac):
    """Bresenham split of n chunks: True -> scalar(ACT), False -> vector(DVE)."""
    out, acc = [], 0.0
    for _ in range(n):
        acc += act_frac
        if acc >= 1.0:
            out.append(True)
            acc -= 1.0
        else:
            out.append(False)
    return out


def build_nc(nt=NT, repeat=1, qw=QW, act_frac=0.47, lbufs=3, skip_dma=False,
             cw=CW, ppbufs=None, pw=None, head=1, nqueues=2, no_drain=0,
             unroll=1, stag=0):
    """pw: psum chunk width (drain granularity); qw: DMA group width."""
    nc = bacc.Bacc("TRN2")
    nq = KS // qw
    if pw is None:
        pw = qw
    npc = qw // pw            # psum chunks per q-group
    if ppbufs is None:
        ppbufs = max(2, 4096 // pw)
    xtd = nc.dram_tensor("xt", [128, NF * B], E4, kind="ExternalInput")
    fqd = nc.dram_tensor("fq", [128, NF * KS], E4, kind="ExternalInput")
    lbd = nc.dram_tensor("lb", [128, nq * nt * qw], E4, kind="ExternalOutput")

    nch = pw // cw
    eng_is_act = _chunk_engines(nq * nt * npc, act_frac)
    with tile.TileContext(nc) as tc:
        with (
            tc.tile_pool(name="fqp", bufs=1) as fqp,
            tc.tile_pool(name="xp", bufs=1) as xp,
            tc.tile_pool(name="pp", bufs=ppbufs, space="PSUM") as pp,
            tc.tile_pool(name="lp", bufs=lbufs) as lp,
        ):
            fq = fqp.tile([128, NF * KS], E4)
            xt = xp.tile([128, NF * B], E4)
            # load x in t-quarters (strided over the f-blocks) so the first
            # matmuls only wait for 1/4 of x plus the first fq group
            xt3w = xt[:].rearrange("p (f b) -> p f b", f=NF)
            xtd3 = xtd[:].rearrange("p (f b) -> p f b", f=NF)
            TQ = B // 4
            gw = NF * qw
            fqw4 = fq[:].rearrange("p (g f k) -> p g f k", g=nq, f=NF)
            fqd4 = fqd[:].rearrange("p (g f k) -> p g f k", g=nq, f=NF)
            hk = qw // 2
            if head == 0:
                nc.sync.dma_start(out=xt3w[:], in_=xtd3[:])
                for g in range(nq):
                    nc.gpsimd.dma_start(out=fq[:, g * gw:(g + 1) * gw],
                                        in_=fqd[:, g * gw:(g + 1) * gw])
            elif head == 1:
                nc.sync.dma_start(out=xt3w[:, :, 0:TQ], in_=xtd3[:, :, 0:TQ])
                nc.gpsimd.dma_start(out=fqw4[:, 0, :, 0:hk], in_=fqd4[:, 0, :, 0:hk])
                nc.gpsimd.dma_start(out=fqw4[:, 0, :, hk:qw], in_=fqd4[:, 0, :, hk:qw])
                for i in range(1, 4):
                    nc.sync.dma_start(out=xt3w[:, :, i * TQ:(i + 1) * TQ],
                                      in_=xtd3[:, :, i * TQ:(i + 1) * TQ])
                for g in range(1, nq):
                    nc.gpsimd.dma_start(out=fq[:, g * gw:(g + 1) * gw],
                                        in_=fqd[:, g * gw:(g + 1) * gw])
            elif head == 2:
                nc.sync.dma_start(out=fqw4[:, 0, :, 0:hk], in_=fqd4[:, 0, :, 0:hk])
                nc.sync.dma_start(out=xt3w[:, :, 0:TQ], in_=xtd3[:, :, 0:TQ])
                nc.sync.dma_start(out=fqw4[:, 0, :, hk:qw], in_=fqd4[:, 0, :, hk:qw])
                for i in range(1, 4):
                    nc.gpsimd.dma_start(out=xt3w[:, :, i * TQ:(i + 1) * TQ],
                                        in_=xtd3[:, :, i * TQ:(i + 1) * TQ])
                for g in range(1, nq):
                    eng = nc.sync if g % 2 == 1 else nc.gpsimd
                    eng.dma_start(out=fq[:, g * gw:(g + 1) * gw],
                                  in_=fqd[:, g * gw:(g + 1) * gw])
            else:
                nc.sync.dma_start(out=fqw4[:, 0, :, 0:hk], in_=fqd4[:, 0, :, 0:hk])
                nc.gpsimd.dma_start(out=xt3w[:, :, 0:TQ], in_=xtd3[:, :, 0:TQ])
                nc.sync.dma_start(out=fqw4[:, 0, :, hk:qw], in_=fqd4[:, 0, :, hk:qw])
                for i in range(1, 4):
                    nc.gpsimd.dma_start(out=xt3w[:, :, i * TQ:(i + 1) * TQ],
                                        in_=xtd3[:, :, i * TQ:(i + 1) * TQ])
                for g in range(1, nq):
                    nc.gpsimd.dma_start(out=fq[:, g * gw:(g + 1) * gw],
                                        in_=fqd[:, g * gw:(g + 1) * gw])
            fq4 = fqw4
            xt3 = xt[:].rearrange("p (f b) -> p f b", f=NF)

            def emit_body(it):
                for q in range(nq):
                    Lq = lp.tile([128, nt * qw], E4, name=f"Lq{it}_{q}",
                                 tag="Lq")
                    for t in range(nt):
                        for pc in range(npc):
                            pt = pp.tile([128, pw], mybir.dt.float32,
                                         name=f"pt{it}_{q}_{t}_{pc}", tag="pt")
                            for c in range(nch):
                                col = pc * pw + c * cw
                                for j in range(0, NF, 2):
                                    nc.tensor.matmul(
                                        pt[:, c * cw:(c + 1) * cw],
                                        xt3[:, j:j + 2, t * 128:(t + 1) * 128],
                                        fq4[:, q, j:j + 2, col:col + cw],
                                        start=(j == 0), stop=(j == NF - 2),
                                        perf_mode=mybir.MatmulPerfMode.DoubleRow,
                                    )
                            if no_drain:
                                continue
                            Lb = Lq[:, t * qw + pc * pw:t * qw + (pc + 1) * pw]
                            if eng_is_act[(q * nt + t) * npc + pc]:
                                nc.scalar.copy(Lb, pt[:])
                            else:
                                nc.vector.tensor_copy(Lb, pt[:])
                    if no_drain:
                        continue
                    if not skip_dma or q == nq - 1:
                        # split each group's DMA so shipping overlaps the
                        # drain; last group finest so the final tail is short
                        queues = ([nc.sync, nc.gpsimd, nc.scalar]
                                  if nqueues > 2 else [nc.sync, nc.gpsimd])
                        nsplit = 8 if q == nq - 1 else (4 if q == nq - 2 else 2)
                        part = nt * qw // nsplit
                        for h in range(nsplit):
                            dmaeng = queues[(2 * q + h) % len(queues)]
                            dmaeng.dma_start(
                                out=lbd[:, q * nt * qw + h * part:
                                        q * nt * qw + (h + 1) * part],
                                in_=Lq[:, h * part:(h + 1) * part])

            trips, tail = divmod(repeat, unroll)
            if trips > 1 or (trips == 1 and tail):
                with tc.For_i(0, trips, 1, staggered_reset=bool(stag)):
                    for u in range(unroll):
                        emit_body(u)
            else:
                tail = repeat
            for u in range(tail):
                emit_body(unroll + u)
    nc.compile()
    return nc


QT = KS // 128            # 64 queue-column tiles per core
BW = 512                  # moving batch chunk (1 psum bank)
NBC = B // BW             # 4 batch chunks


def build_nc2(nt=NT, repeat=1, act_frac=0.55, lbufs=3, ppbufs=2, dgrp=1,
              nqueues=2, drain_chunks=2, qt_limit=0,
              no_mm=0, no_drain=0, no_dma=0, mm_order=0):
    """v2: feature_queue stationary, x moving.  Each weight load (one
    [128,2,128] fq tile) is amortized over NBC consecutive matmuls that
    stream the whole batch; j-pairs accumulate in psum.  Output layout is
    logits.T: lb[p, qt*B + b] = logits[b, qt*128 + p].

    dgrp: queue-tiles per DMA group."""
    nc = bacc.Bacc("TRN2")
    xtd = nc.dram_tensor("xt", [128, NF * B], E4, kind="ExternalInput")
    fqd = nc.dram_tensor("fq", [128, QT * NF * 128], E4, kind="ExternalInput")
    lbd = nc.dram_tensor("lb", [128, QT * B], E4, kind="ExternalOutput")

    eng_is_act = _chunk_engines(QT * drain_chunks, act_frac)
    dw = B // drain_chunks
    with tile.TileContext(nc) as tc:
        with (
            tc.tile_pool(name="fqp", bufs=1) as fqp,
            tc.tile_pool(name="xp", bufs=1) as xp,
            tc.tile_pool(name="pp", bufs=ppbufs, space="PSUM") as pp,
            tc.tile_pool(name="lp", bufs=lbufs) as lp,
        ):
            fq = fqp.tile([128, QT * NF * 128], E4)
            xt = xp.tile([128, NF * B], E4)
            fqv = fq[:].rearrange("p (q j s m) -> p q j s m", q=QT, j=2, s=2)
            fqdv = fqd[:].rearrange("p (q j s m) -> p q j s m", q=QT, j=2, s=2)
            xtv = xt[:].rearrange("p (j s b) -> p j s b", j=2, s=2)
            xtdv = xtd[:].rearrange("p (j s b) -> p j s b", j=2, s=2)
            # head: x first (small), then fq by groups so matmuls start early
            nc.sync.dma_start(out=xtv[:], in_=xtdv[:])
            HG = 8                       # fq load groups
            gq = QT // HG
            for g in range(HG):
                eng = nc.gpsimd if g % 2 == 0 else nc.sync
                eng.dma_start(out=fqv[:, g * gq:(g + 1) * gq],
                              in_=fqdv[:, g * gq:(g + 1) * gq])

            if repeat > 1:
                loop_ctx = tc.For_i(0, repeat, 1)
                loop_ctx.__enter__()
            for qt in range(qt_limit or QT):
                pt = pp.tile([128, B], mybir.dt.float32, name=f"pt{qt}", tag="pt")
                if not no_mm:
                    order = ([(jp, bc) for jp in range(2) for bc in range(NBC)]
                             if mm_order == 0 else
                             [(jp, bc) for bc in range(NBC) for jp in range(2)])
                    for jp, bc in order:
                        lhsT = fqv[:, qt, jp]                  # [128, 2, 128]
                        rhs = xtv[:, jp, :, bc * BW:(bc + 1) * BW]
                        nc.tensor.matmul(
                            pt[:, bc * BW:(bc + 1) * BW], lhsT, rhs,
                            start=(jp == 0), stop=(jp == 1),
                            perf_mode=mybir.MatmulPerfMode.DoubleRow,
                        )
                Lt = lp.tile([128, B], E4, name=f"Lt{qt}", tag="Lt")
                if no_drain:
                    continue
                for dc in range(drain_chunks):
                    sl = slice(dc * dw, (dc + 1) * dw)
                    if eng_is_act[qt * drain_chunks + dc]:
                        nc.scalar.copy(Lt[:, sl], pt[:, sl])
                    else:
                        nc.vector.tensor_copy(Lt[:, sl], pt[:, sl])
                if no_dma:
                    continue
                queues = ([nc.sync, nc.gpsimd, nc.scalar]
                          if nqueues > 2 else [nc.sync, nc.gpsimd])
                dmaeng = queues[qt % len(queues)]
                dmaeng.dma_start(out=lbd[:, qt * B:(qt + 1) * B], in_=Lt[:])
            if repeat > 1:
                loop_ctx.__exit__(None, None, None)
    nc.compile()
    return nc


def _pack_inputs2(x, feature_queue):
    e4 = mybir.dt.np(E4)
    xq = x.astype(e4)
    arr = np.ascontiguousarray(xq.T).reshape(2, 2, 128, B)
    xT = np.ascontiguousarray(arr.transpose(2, 0, 1, 3)).reshape(128, NF * B)
    fq_packs = []
    for c in range(NCORES):
        shard = feature_queue[c * KS:(c + 1) * KS].astype(e4)    # [KS, F]
        a = np.ascontiguousarray(shard.T).reshape(2, 2, 128, QT, 128)
        fq_packs.append(np.ascontiguousarray(
            a.transpose(2, 3, 0, 1, 4)).reshape(128, QT * NF * 128))
    return xT, fq_packs


def _assemble_logits2(results):
    cols = []
    for r in results:
        lb = np.asarray(r["lb"]).reshape(128, QT, B)
        cols.append(np.ascontiguousarray(
            lb.transpose(2, 1, 0)).reshape(B, KS).astype(np.float32))
    return np.concatenate(cols, axis=1)


def _pack_inputs(x, feature_queue, qw=QW):
    """fp8-quantize and pack [*, F] operands as [128, NF * n] f-blocked."""
    e4 = mybir.dt.np(E4)
    xT = np.ascontiguousarray(
        x.T.reshape(NF, 128, B).transpose(1, 0, 2).reshape(128, NF * B)).astype(e4)
    fq_packs = []
    G = KS // qw
    for c in range(NCORES):
        shard = feature_queue[c * KS:(c + 1) * KS]      # [KS, F]
        fqT = np.ascontiguousarray(
            shard.T.reshape(NF, 128, G, qw).transpose(1, 2, 0, 3)
            .reshape(128, NF * KS)).astype(e4)
        fq_packs.append(fqT)
    return xT, fq_packs


def _assemble_logits(results, qw=QW):
    """[core][128, NQ*NT*QW] fp8 -> [B, K] float32.
    lb[p, ((q*NT)+t)*qw + c] == logits[t*128+p, q*qw+c]."""
    nq = KS // qw
    cols = []
    for r in results:
        lb = np.asarray(r["lb"])                       # [128, NQ*NT*QW] fp8
        lb = lb.reshape(128, nq, NT, qw).transpose(2, 0, 1, 3).reshape(B, KS)
        cols.append(lb.astype(np.float32))
    return np.concatenate(cols, axis=1)                # [B, K] f32


def _exact_pick(x, feature_queue, approx, mode):
    """Exact argmax/argmin: rescore all candidates within MARGIN of the
    approx extreme with an fp64 dot; ties -> smallest index."""
    if mode == "max":
        ext = approx.max(axis=1, keepdims=True)
        rows, cands = np.nonzero(approx >= ext - MARGIN)
    else:
        ext = approx.min(axis=1, keepdims=True)
        rows, cands = np.nonzero(approx <= ext + MARGIN)
    scores = np.einsum("if,if->i", x[rows].astype(np.float64),
                       feature_queue[cands].astype(np.float64))
    out = np.empty(B, dtype=np.int64)
    starts = np.searchsorted(rows, np.arange(B))
    ends = np.searchsorted(rows, np.arange(B), side="right")
    for b in range(B):
        s, e = starts[b], ends[b]
        sc = scores[s:e]
        ks = cands[s:e]
        top = sc.max() if mode == "max" else sc.min()
        out[b] = ks[sc == top].min()
    return out


# Selected pipeline: "base" or "v2" (build/pack/assemble triplet used by
# kernel() and by test.py's timing).
VARIANT = "base"
BEST_KW = dict(act_frac=0.54, unroll=8)


def build_best(repeat=1):
    if VARIANT == "v2":
        return build_nc2(NT, repeat)
    return build_nc(NT, repeat, **BEST_KW)


def pack_best(x, feature_queue):
    if VARIANT == "v2":
        return _pack_inputs2(x, feature_queue)
    return _pack_inputs(x, feature_queue)


def assemble_best(results):
    if VARIANT == "v2":
        return _assemble_logits2(results)
    return _assemble_logits(results)


def kernel(x, feature_queue, rep_queue):
    global _nc_cache
    x = np.asarray(x, dtype=np.float32)
    feature_queue = np.asarray(feature_queue, dtype=np.float32)
    rep_queue = np.asarray(rep_queue, dtype=np.float32)

    if _nc_cache is None:
        _nc_cache = build_best()
    nc = _nc_cache

    xT, fq_packs = pack_best(x, feature_queue)
    in_maps = [{"xt": xT, "fq": fq_packs[c]} for c in range(NCORES)]
    results = run_bass_kernel_spmd(nc, in_maps, core_ids=list(range(NCORES))).results

    approx = assemble_best(results)
    pos_idx = _exact_pick(x, feature_queue, approx, "max")
    neg_idx = _exact_pick(x, feature_queue, approx, "min")
    return (rep_queue[pos_idx], rep_queue[neg_idx])

